# revision 12
# baseline (speedup 1.0000x reference)
"""Trainium2 Bass kernel for a (buggy-but-well-defined) ConvTranspose2d.

Math (matches the reference exactly):
  out[b, co, i, j] = sum_{ci,kh,kw} ker[ci,co,3-kh,3-kw] * xpad[b,ci,i+kh,j+kw]
                     + bias_sum * cnt[i] * cnt[j]          for i,j in [0,66)
  out is zero elsewhere in the (B,128,126,126) output.
  xpad = x[:, :, :63, :63] zero-padded by 3 on every side.
  cnt  = conv(ones(63), ones(4)) = [1,2,3,4,...,4,3,2,1]  (len 66)

Strategy: data-parallel over batch (2 items / core on 8 cores), bf16.
Per core, per image, 10 groups of <=7 output rows; each group accumulates
its 16 shifted 128x128xN matmuls (contraction over ci) into one PSUM bank.
The PE stream is pure bf16 (1 col/cycle, fast FWL weight loads).  x is
shipped with horizontal padding only; each group's first matmul (a
full-row-coverage tap) covers the whole PSUM region with start=True, and
every other tap is trimmed to the rows AND columns that touch real data
(per-element PSUM has_written bits make the partial accumulation
well-defined).  The bias field is replicated across all 128 partitions
on-chip by ten K=1 bf16 matmuls (ones[1,128].T @ field[1,N]) that run
first, while the x DMA is still in flight — they double as the PE
warm-up for the HAM clock gate.  Group close = one DVE tensor_add (PSUM
+ bias -> bf16 out tile) followed by the output DMA; the very last close
is split in two so its DMA overlaps the final add.  All input DMAs ride
one queue (scalar) in consumption order; sync carries the output DMAs.
The mostly-zero full output is assembled host-side.
"""

import ml_dtypes
import numpy as np

import concourse.bacc as bacc
import concourse.mybir as mybir
import concourse.tile as tile
from concourse.bass_utils import run_bass_kernel_spmd

B, CIN, COUT, K, H, W = 16, 128, 128, 4, 64, 64
NCORES = 8
BPC = B // NCORES          # batch items per core
HV = H - 1                 # 63 valid input rows/cols
RS = HV + 2 * (K - 1)      # 69: row stride (cols padded by 3 each side)
HO = HV + K - 1            # 66 output rows/cols (nonzero region)
HOUT = (H - 1) * 2         # 126 full output rows/cols
NWT = K * K * COUT         # 2048 weight cols
NXI = HV * RS              # 4347 cols per image (63 rows x 69 padded cols)
NXW = NWT + BPC * NXI
NBF = HO * HO + COUT       # bias-field input: 66*66 field + 128 ones
F32 = mybir.dt.float32
BF16 = mybir.dt.bfloat16

GROUPS = [(0, 7), (7, 7), (14, 7), (21, 7), (28, 5),
          (33, 7), (40, 7), (47, 7), (54, 7), (61, 5)]

# Tap layout order in the weight tensor: kh=3 first so group 0 (whose
# full-coverage tap is kh=3) can start on the first small weight chunk.
KH_LAYOUT = [3, 0, 1, 2]
TAP_COL = {}
for _i, _kh in enumerate(KH_LAYOUT):
    for _kw in range(K):
        TAP_COL[(_kh, _kw)] = (_i * K + _kw) * COUT

_CACHE = {}


def _kh_order(i0, r):
    """Tap row order for a group: a full-row-coverage kh first (its kw=0
    matmul carries start=True and must clear the whole PSUM region)."""
    def full(kh):
        return 3 - kh - i0 <= 0 and 66 - kh - i0 >= r
    return sorted(range(K), key=lambda kh: not full(kh))


def _build_nc():
    # Bacc (not raw Bass): its finalize() legalizes sync waits — moving
    # excess matmul waits onto LDWEIGHTS and splitting multi-waits onto
    # EventSemaphore instructions — which walrus codegen requires.
    nc = bacc.Bacc(None)
    xw = nc.dram_tensor("xw", [CIN, NXW], BF16, kind="ExternalInput")
    bf = nc.dram_tensor("bf", [NBF], BF16, kind="ExternalInput")
    out = nc.dram_tensor("out", [BPC, COUT, HO, HO], BF16,
                         kind="ExternalOutput")

    with tile.TileContext(nc) as tc:
        with (
            tc.tile_pool(name="xwpool", bufs=1) as xwpool,
            tc.tile_pool(name="cpool", bufs=1) as cpool,
            tc.tile_pool(name="bspool", bufs=1) as bspool,
            tc.tile_pool(name="acc", bufs=6, space="PSUM") as psum_pool,
            tc.tile_pool(name="opool", bufs=4) as opool,
        ):
            # Bias field + ones: tiny, first in the DMA queue.
            bft = cpool.tile([1, NBF], BF16)
            nc.scalar.dma_start(bft[:1, :], bf[None, :])

            xwt = xwpool.tile([CIN, NXW], BF16)
            xv = xwt[:, NWT:].rearrange("p (b r c) -> p b r c",
                                        b=BPC, r=HV, c=RS)
            xwsrc = xw[:, NWT:].rearrange("p (b r c) -> p b r c",
                                          b=BPC, r=HV, c=RS)

            # Input DMAs on one queue (scalar) in consumption order;
            # chunks kept coarse so per-partition lines stay multi-KB.
            def xchunk(b, r0, nr):
                nc.scalar.dma_start(xv[:, b, r0:r0 + nr, :],
                                    xwsrc[:, b, r0:r0 + nr, :])

            nc.scalar.dma_start(xwt[:, :4 * COUT], xw[:, :4 * COUT])
            xchunk(0, 0, 7)
            nc.scalar.dma_start(xwt[:, 4 * COUT:NWT], xw[:, 4 * COUT:NWT])
            xchunk(0, 7, 14)
            xchunk(0, 21, 42)
            xchunk(1, 0, 63)

            # Replicate the bias field across all 128 partitions:
            # ones[1,128].T @ field[1, N] per group.  These run while the
            # x DMA is in flight and warm up the HAM clock gate.
            bias_sb = bspool.tile([COUT, HO * HO], BF16)
            ones = bft[0:1, HO * HO:]
            for i0, r in GROUPS:
                bp = psum_pool.tile([COUT, 462], F32, tag="acc", name="acc")
                nc.tensor.matmul(bp[:, :r * HO], ones,
                                 bft[0:1, i0 * HO:(i0 + r) * HO],
                                 start=True, stop=True)
                nc.vector.tensor_copy(bias_sb[:, i0 * HO:(i0 + r) * HO],
                                      bp[:, :r * HO])

            # Main conv stream: groups outer, taps inner.  The first tap
            # (full row coverage, kw=0) writes the whole PSUM region with
            # start=True; all other taps are trimmed to real-data rows
            # (di0:di1) and columns (63 of 66) and accumulate into a 2D
            # row/col window of the bank.
            for b in range(BPC):
                for i0, r in GROUPS:
                    acc = psum_pool.tile([COUT, 462], F32,
                                         tag="acc", name="acc")
                    av = acc.rearrange("p (r c) -> p r c", r=7, c=HO)
                    order = _kh_order(i0, r)
                    for kh in order:
                        di0 = max(0, 3 - kh - i0)
                        di1 = min(r, 66 - kh - i0)
                        x0 = i0 + kh - 3 + di0
                        for kw in range(K):
                            lhsT = xwt[:, TAP_COL[(kh, kw)]:
                                       TAP_COL[(kh, kw)] + COUT]
                            first = kh == order[0] and kw == 0
                            last = kh == order[-1] and kw == K - 1
                            if first:
                                rhs = xv[:, b, x0:x0 + r, 0:HO]
                                dst = acc[:, :r * HO]
                            else:
                                c0 = max(0, 3 - kw)
                                rhs = xv[:, b, x0:x0 + di1 - di0, 3:3 + HV]
                                dst = av[:, di0:di1, c0:c0 + HV]
                            nc.tensor.matmul(dst, lhsT, rhs,
                                             start=first, stop=last)
                    # Close: fuse the bias add into the PSUM drain.  The
                    # very last close is split so its first output DMA
                    # overlaps the second tensor_add.
                    final = b == BPC - 1 and i0 == GROUPS[-1][0]
                    splits = [(0, 3), (3, r - 3)] if final else [(0, r)]
                    for s0, sr in splits:
                        otile = opool.tile([COUT, 462], BF16,
                                           tag="ot", name="ot")
                        nc.vector.tensor_add(
                            otile[:, :sr * HO],
                            acc[:, s0 * HO:(s0 + sr) * HO],
                            bias_sb[:, (i0 + s0) * HO:(i0 + s0 + sr) * HO])
                        nc.sync.dma_start(out[b, :, i0 + s0:i0 + s0 + sr, :],
                                          otile[:, :sr * HO])
    nc.finalize()
    return nc


def get_nc():
    if "nc" not in _CACHE:
        _CACHE["nc"] = _build_nc()
    return _CACHE["nc"]


def prep_inputs(x, kernel, bias):
    """Host-side prep: per-core input maps (numpy only, negligible cost)."""
    x = np.asarray(x, dtype=np.float32)
    ker = np.asarray(kernel, dtype=np.float32)
    bias = np.asarray(bias, dtype=np.float32)

    kf = ker[:COUT, :, ::-1, ::-1]                    # [ci, co, kh, kw] flipped
    wt = np.empty((CIN, NWT), ml_dtypes.bfloat16)
    for kh in range(K):
        for kw in range(K):
            c = TAP_COL[(kh, kw)]
            wt[:, c:c + COUT] = kf[:, :, kh, kw].astype(ml_dtypes.bfloat16)

    cnt = np.convolve(np.ones(HV, np.float32), np.ones(K, np.float32))
    bias_sum = np.sum(bias[:COUT], dtype=np.float32)
    bfield = np.empty(NBF, np.float32)
    bfield[:HO * HO] = (bias_sum * np.outer(cnt, cnt)).astype(np.float32).ravel()
    bfield[HO * HO:] = 1.0
    bfield = bfield.astype(ml_dtypes.bfloat16)

    xb = x[:, :, :HV, :HV].astype(ml_dtypes.bfloat16)
    in_maps = []
    for c in range(NCORES):
        xwm = np.zeros((CIN, NXW), ml_dtypes.bfloat16)
        xwm[:, :NWT] = wt
        xp = xwm[:, NWT:].reshape(CIN, BPC, HV, RS)
        xp[:, :, :, K - 1:K - 1 + HV] = \
            xb[c * BPC:(c + 1) * BPC].transpose(1, 0, 2, 3)
        in_maps.append({"xw": xwm, "bf": bfield})
    return in_maps


def assemble(per_core_outs):
    out = np.zeros((B, COUT, HOUT, HOUT), np.float32)
    for c, o in enumerate(per_core_outs):
        out[c * BPC:(c + 1) * BPC, :, :HO, :HO] = np.asarray(o, np.float32)
    return out


def run(inputs, **spmd_kwargs):
    """Returns (full_output, BassKernelResults)."""
    nc = get_nc()
    in_maps = prep_inputs(**inputs)
    res = run_bass_kernel_spmd(nc, in_maps, list(range(NCORES)), **spmd_kwargs)
    return assemble([r["out"] for r in res.results]), res


def kernel(**inputs):
    out, _ = run(inputs)
    return out


# revision 16
# speedup vs baseline: 1.0181x; 1.0181x over previous
"""Trainium2 Bass kernel for a (buggy-but-well-defined) ConvTranspose2d.

Math (matches the reference exactly):
  out[b, co, i, j] = sum_{ci,kh,kw} ker[ci,co,3-kh,3-kw] * xpad[b,ci,i+kh,j+kw]
                     + bias_sum * cnt[i] * cnt[j]          for i,j in [0,66)
  out is zero elsewhere in the (B,128,126,126) output.
  xpad = x[:, :, :63, :63] zero-padded by 3 on every side.
  cnt  = conv(ones(63), ones(4)) = [1,2,3,4,...,4,3,2,1]  (len 66)

Strategy: data-parallel over batch (2 items / core on 8 cores), bf16.
Per core, per image, 10 groups of <=7 output rows; each group accumulates
its 16 shifted 128x128xN matmuls (contraction over ci) into one PSUM bank.
The PE stream is pure bf16 (1 col/cycle, fast FWL weight loads).  x is
shipped with horizontal padding only; each group's first matmul (a
full-row-coverage tap) covers the whole PSUM region with start=True, and
every other tap is trimmed to the rows AND columns that touch real data
(per-element PSUM has_written bits make the partial accumulation
well-defined).  The bias field is replicated across all 128 partitions
on-chip by ten K=1 bf16 matmuls (ones[1,128].T @ field[1,N]) that run
first, while the x DMA is still in flight — they double as the PE
warm-up for the HAM clock gate.  Group close = one DVE tensor_add (PSUM
+ bias -> bf16 out tile) followed by the output DMA; the very last close
is split in two so its DMA overlaps the final add.  All input DMAs ride
one queue (scalar) in consumption order; sync carries the output DMAs.
The mostly-zero full output is assembled host-side.
"""

import ml_dtypes
import numpy as np

import concourse.bacc as bacc
import concourse.mybir as mybir
import concourse.tile as tile
from concourse.bass_utils import run_bass_kernel_spmd

B, CIN, COUT, K, H, W = 16, 128, 128, 4, 64, 64
NCORES = 8
BPC = B // NCORES          # batch items per core
HV = H - 1                 # 63 valid input rows/cols
RS = HV + 2 * (K - 1)      # 69: row stride (cols padded by 3 each side)
HO = HV + K - 1            # 66 output rows/cols (nonzero region)
HOUT = (H - 1) * 2         # 126 full output rows/cols
NWT = K * K * COUT         # 2048 weight cols
NXI = HV * RS              # 4347 cols per image (63 rows x 69 padded cols)
NXW = NWT + BPC * NXI
NBF = HO * HO              # bias-field input: 66*66 field (1 partition)
NWARM = 6                  # junk warm-up matmuls (HAM clock-gate)
F32 = mybir.dt.float32
BF16 = mybir.dt.bfloat16

GROUPS = [(0, 7), (7, 7), (14, 7), (21, 7), (28, 5),
          (33, 7), (40, 7), (47, 7), (54, 7), (61, 5)]

# Tap layout order in the weight tensor: kh=3 first so group 0 (whose
# full-coverage tap is kh=3) can start on the first small weight chunk.
KH_LAYOUT = [3, 0, 1, 2]
TAP_COL = {}
for _i, _kh in enumerate(KH_LAYOUT):
    for _kw in range(K):
        TAP_COL[(_kh, _kw)] = (_i * K + _kw) * COUT

_CACHE = {}


def _kh_order(i0, r):
    """Tap row order for a group: a full-row-coverage kh first (its kw=0
    matmul carries start=True and must clear the whole PSUM region)."""
    def full(kh):
        return 3 - kh - i0 <= 0 and 66 - kh - i0 >= r
    return sorted(range(K), key=lambda kh: not full(kh))


def _build_nc():
    # Bacc (not raw Bass): its finalize() legalizes sync waits — moving
    # excess matmul waits onto LDWEIGHTS and splitting multi-waits onto
    # EventSemaphore instructions — which walrus codegen requires.
    nc = bacc.Bacc(None)
    xw = nc.dram_tensor("xw", [CIN, NXW], BF16, kind="ExternalInput")
    bf = nc.dram_tensor("bf", [NBF], BF16, kind="ExternalInput")
    out = nc.dram_tensor("out", [BPC, COUT, HO, HO], BF16,
                         kind="ExternalOutput")

    with tile.TileContext(nc) as tc:
        with (
            tc.tile_pool(name="xwpool", bufs=1) as xwpool,
            tc.tile_pool(name="cpool", bufs=1) as cpool,
            tc.tile_pool(name="bspool", bufs=1) as bspool,
            tc.tile_pool(name="warm", bufs=1) as warmpool,
            tc.tile_pool(name="acc", bufs=6, space="PSUM") as psum_pool,
            tc.tile_pool(name="opool", bufs=4) as opool,
        ):
            # PE warm-up fodder: zeros, no DMA dependency.  Full-K junk
            # matmuls are the only work that trips the HAM clock gate
            # (K=1 matmuls barely register as PE activity).
            warmt = warmpool.tile([CIN, 462], BF16)
            nc.gpsimd.memset(warmt, 0.0)

            # Bias field: tiny, first in the DMA queue.
            bft = cpool.tile([1, NBF], BF16)
            nc.scalar.dma_start(bft[:1, :], bf[None, :])

            xwt = xwpool.tile([CIN, NXW], BF16)
            xv = xwt[:, NWT:].rearrange("p (b r c) -> p b r c",
                                        b=BPC, r=HV, c=RS)
            xwsrc = xw[:, NWT:].rearrange("p (b r c) -> p b r c",
                                          b=BPC, r=HV, c=RS)

            # Input DMAs on one queue (scalar) in consumption order;
            # chunks kept coarse so per-partition lines stay multi-KB.
            def xchunk(b, r0, nr):
                nc.scalar.dma_start(xv[:, b, r0:r0 + nr, :],
                                    xwsrc[:, b, r0:r0 + nr, :])

            nc.scalar.dma_start(xwt[:, :4 * COUT], xw[:, :4 * COUT])
            xchunk(0, 0, 7)
            nc.scalar.dma_start(xwt[:, 4 * COUT:NWT], xw[:, 4 * COUT:NWT])
            xchunk(0, 7, 14)
            xchunk(0, 21, 42)
            xchunk(1, 0, 63)

            # Replicate the bias field across all 128 partitions on the
            # (otherwise idle) GpSimd engine — zero PE/DVE cost.
            bias_sb = bspool.tile([COUT, HO * HO], BF16)
            nc.gpsimd.partition_broadcast(bias_sb, bft[0:1, :])

            # Junk matmuls: keep the PE busy from engine-up until the
            # first image chunk lands, so the HAM clock gate is released
            # when real work starts.
            wps = psum_pool.tile([COUT, 462], F32, tag="acc", name="acc")
            for _ in range(NWARM):
                nc.tensor.matmul(wps, warmt[:, :CIN], warmt[:, :462],
                                 start=True, stop=True)

            # Main conv stream: groups outer, taps inner.  The first tap
            # (full row coverage, kw=0) writes the whole PSUM region with
            # start=True; all other taps are trimmed to real-data rows
            # (di0:di1) and columns (63 of 66) and accumulate into a 2D
            # row/col window of the bank.
            for b in range(BPC):
                for i0, r in GROUPS:
                    acc = psum_pool.tile([COUT, 462], F32,
                                         tag="acc", name="acc")
                    av = acc.rearrange("p (r c) -> p r c", r=7, c=HO)
                    order = _kh_order(i0, r)
                    for kh in order:
                        di0 = max(0, 3 - kh - i0)
                        di1 = min(r, 66 - kh - i0)
                        x0 = i0 + kh - 3 + di0
                        for kw in range(K):
                            lhsT = xwt[:, TAP_COL[(kh, kw)]:
                                       TAP_COL[(kh, kw)] + COUT]
                            first = kh == order[0] and kw == 0
                            last = kh == order[-1] and kw == K - 1
                            if first:
                                rhs = xv[:, b, x0:x0 + r, 0:HO]
                                dst = acc[:, :r * HO]
                            else:
                                c0 = max(0, 3 - kw)
                                rhs = xv[:, b, x0:x0 + di1 - di0, 3:3 + HV]
                                dst = av[:, di0:di1, c0:c0 + HV]
                            nc.tensor.matmul(dst, lhsT, rhs,
                                             start=first, stop=last)
                    # Close: fuse the bias add into the PSUM drain.  The
                    # very last close is split so its first output DMA
                    # overlaps the second tensor_add.
                    final = b == BPC - 1 and i0 == GROUPS[-1][0]
                    splits = [(0, 3), (3, r - 3)] if final else [(0, r)]
                    for s0, sr in splits:
                        otile = opool.tile([COUT, 462], BF16,
                                           tag="ot", name="ot")
                        nc.vector.tensor_add(
                            otile[:, :sr * HO],
                            acc[:, s0 * HO:(s0 + sr) * HO],
                            bias_sb[:, (i0 + s0) * HO:(i0 + s0 + sr) * HO])
                        nc.sync.dma_start(out[b, :, i0 + s0:i0 + s0 + sr, :],
                                          otile[:, :sr * HO])
    nc.finalize()
    return nc


def get_nc():
    if "nc" not in _CACHE:
        _CACHE["nc"] = _build_nc()
    return _CACHE["nc"]


def prep_inputs(x, kernel, bias):
    """Host-side prep: per-core input maps (numpy only, negligible cost)."""
    x = np.asarray(x, dtype=np.float32)
    ker = np.asarray(kernel, dtype=np.float32)
    bias = np.asarray(bias, dtype=np.float32)

    kf = ker[:COUT, :, ::-1, ::-1]                    # [ci, co, kh, kw] flipped
    wt = np.empty((CIN, NWT), ml_dtypes.bfloat16)
    for kh in range(K):
        for kw in range(K):
            c = TAP_COL[(kh, kw)]
            wt[:, c:c + COUT] = kf[:, :, kh, kw].astype(ml_dtypes.bfloat16)

    cnt = np.convolve(np.ones(HV, np.float32), np.ones(K, np.float32))
    bias_sum = np.sum(bias[:COUT], dtype=np.float32)
    bfield = (bias_sum * np.outer(cnt, cnt)).ravel().astype(ml_dtypes.bfloat16)

    xb = x[:, :, :HV, :HV].astype(ml_dtypes.bfloat16)
    in_maps = []
    for c in range(NCORES):
        xwm = np.zeros((CIN, NXW), ml_dtypes.bfloat16)
        xwm[:, :NWT] = wt
        xp = xwm[:, NWT:].reshape(CIN, BPC, HV, RS)
        xp[:, :, :, K - 1:K - 1 + HV] = \
            xb[c * BPC:(c + 1) * BPC].transpose(1, 0, 2, 3)
        in_maps.append({"xw": xwm, "bf": bfield})
    return in_maps


def assemble(per_core_outs):
    out = np.zeros((B, COUT, HOUT, HOUT), np.float32)
    for c, o in enumerate(per_core_outs):
        out[c * BPC:(c + 1) * BPC, :, :HO, :HO] = np.asarray(o, np.float32)
    return out


def run(inputs, **spmd_kwargs):
    """Returns (full_output, BassKernelResults)."""
    nc = get_nc()
    in_maps = prep_inputs(**inputs)
    res = run_bass_kernel_spmd(nc, in_maps, list(range(NCORES)), **spmd_kwargs)
    return assemble([r["out"] for r in res.results]), res


def kernel(**inputs):
    out, _ = run(inputs)
    return out
